# revision 16
# baseline (speedup 1.0000x reference)
"""Talking-heads attention kernel for Trainium2, 8 NeuronCores.

Problem: B=4, N=2048, DIM=512, H=8, DH=64 talking-heads attention
(qkv proj -> per-head scores -> th1 head-mix -> softmax -> th2 head-mix
 -> attn @ v -> out proj).

Sharding: data-parallel over (batch, query-half): core c handles batch c//2,
query rows [1024*(c%2), 1024*(c%2)+1024).  Communication-free.  The host
permutes each core's x so its query half comes first; key order is
attention-invariant, so K/V use the permuted order untouched.

Device pipeline per core:
  1. x -> bf16 -> xbar-transpose (pairs of tiles per transpose) -> x^T;
     QKV projection on TensorE.  K^T and the th1-fused interleaved Qhat are
     evicted straight to fp8e4 (scores run fp8 DoubleRow, 256-deep chains);
     the softmax scale 1/sqrt(dh) is applied inside the exp activation.
  2. Dummy warm-up matmuls during the input DMA keep the PE HAM clock at
     full rate; V-projection matmuls are deferred into the idle @V slots of
     main-loop iteration 0 (vfill).
  3. Per 128-query tile: fused scores land in interleaved PSUM; exp on
     ScalarE with fused per-row accumulation; softmax division folded into
     the second mix's stationary weights (th2[g,h]/Z row scaling).
  4. The Z->reciprocal->l2 chain is emitted right after each m2 (two
     iterations ahead of use) so it never head-of-line blocks the Vector
     queue; fscore(k+2) is emitted before m2(k) so the PE always has
     runway while that chain completes.
  5. mix2 output is xbar-DMA-transposed to key-major A^T, attn@V runs with
     V as the stationary operand, followed by the output projection.
"""

import sys

sys.path.insert(0, "/opt/trn_rl_repo")

import numpy as np
import ml_dtypes

import concourse.bass as bass
from concourse import bacc
import concourse.mybir as mybir
import concourse.tile as tile
from concourse.bass_utils import run_bass_kernel_spmd

BF16 = mybir.dt.bfloat16
F32 = mybir.dt.float32
F8 = mybir.dt.float8e4
AF = mybir.ActivationFunctionType
DR = mybir.MatmulPerfMode.DoubleRow

FP8 = True            # fp8e4 + DoubleRow score matmuls

B, N, DIM = 4, 2048, 512
H, DH = 8, 64
NCORES = 8
NLOC = N // 2          # query rows per core
NT = NLOC // 128       # 8 query tiles per core
MT = N // 128          # 16 key chunks
NG = 16                # n16 group size in interleaved tiles
SCALE = DH ** -0.5


def build_nc():
    nc = bacc.Bacc()

    x = nc.declare_dram_parameter("x", [N, DIM], BF16, isOutput=False)
    wq = nc.declare_dram_parameter("wq", [DIM, DIM], BF16, isOutput=False)
    wk = nc.declare_dram_parameter("wk", [DIM, DIM], BF16, isOutput=False)
    wv = nc.declare_dram_parameter("wv", [DIM, DIM], BF16, isOutput=False)
    wo = nc.declare_dram_parameter("wo", [DIM, DIM], BF16, isOutput=False)
    th1s = nc.declare_dram_parameter("th1s", [128, 32], F32, isOutput=False)
    t2t = nc.declare_dram_parameter("t2t", [128, 128], BF16, isOutput=False)
    # rows: bq, bk, bv, bo (bq unscaled; scale applied in exp)
    bia = nc.declare_dram_parameter("bia", [1, 4 * DIM], BF16, isOutput=False)
    # per-partition q/k bias: col rc -> bq[128rc+p], col 4+rc -> bk[128rc+p]
    biaqk = nc.declare_dram_parameter("biaqk", [128, 8], F32, isOutput=False)
    y = nc.declare_dram_parameter("y", [NLOC, DIM], F32, isOutput=True)

    SD = F8 if FP8 else BF16

    with tile.TileContext(nc) as tc:
        with (
            tc.tile_pool(name="pw", bufs=1) as pw,      # persistent across phases
            tc.tile_pool(name="psA", bufs=3, space="PSUM") as psA,  # [128,1024] mixes
            tc.tile_pool(name="psB", bufs=2, space="PSUM") as psB,  # [128,512] scores/proj/@V
        ):
            # persistent small tensors
            wo_sb = [pw.tile([128, DIM], BF16, name=f"wo{i}", tag=f"wo{i}") for i in range(4)]
            for i in range(4):
                nc.gpsimd.dma_start(wo_sb[i][:], wo[128 * i:128 * (i + 1), :])
            th1s_sb = pw.tile([128, 32], F32, name="th1s", tag="th1s")
            t2_sb = pw.tile([128, 128], BF16, name="t2", tag="t2")
            nc.gpsimd.dma_start(th1s_sb[:], th1s[:])
            nc.gpsimd.dma_start(t2_sb[:], t2t[:])
            bia_sb = pw.tile([1, 4 * DIM], BF16, name="bia", tag="bia")
            nc.gpsimd.dma_start(bia_sb[:], bia[:])
            biaqk_sb = pw.tile([128, 8], F32, name="biaqk", tag="biaqk")
            nc.gpsimd.dma_start(biaqk_sb[:], biaqk[:])
            ones_sb = pw.tile([1, DIM], BF16, name="ones", tag="ones")
            nc.any.memset(ones_sb[:], 1.0)

            # persistent activations: th1-fused interleaved Qhat, K^T, V.
            # qhat[p=(h,d) in chunk rc, col (j, g, n16)] = th1[g,h] * Q^T[(h,d), n]
            # with n = 16*j + n16 (local query index).  For fp8 the four rc
            # chunks live as two DoubleRow pair tiles [128, ko=2, cols].
            if FP8:
                qhat_sb = [pw.tile([128, 2, NLOC * 8], SD, name=f"qh{i}", tag=f"qh{i}")
                           for i in range(2)]
                kt_sb = [pw.tile([128, 2, N], SD, name=f"kt{i}", tag=f"kt{i}")
                         for i in range(2)]

                def qhat_dst(rc):
                    return qhat_sb[rc // 2][:, rc % 2, :]

                def kt_dst(rc, c0, c1):
                    return kt_sb[rc // 2][:, rc % 2, c0:c1]
            else:
                qhat_sb = [pw.tile([128, NLOC * 8], SD, name=f"qh{i}", tag=f"qh{i}")
                           for i in range(4)]
                kt_sb = [pw.tile([128, N], SD, name=f"kt{i}", tag=f"kt{i}")
                         for i in range(4)]

                def qhat_dst(rc):
                    return qhat_sb[rc][:]

                def kt_dst(rc, c0, c1):
                    return kt_sb[rc][:, c0:c1]

            v_sb = pw.tile([128, MT, DIM], BF16, name="v", tag="v")
            # xt/wv persist into iteration 0 (vfill) so they live in pw
            # xt_sb[p, t, j, f]: dim = j*128+p, key row m = t*128+f
            xt_sb = pw.tile([128, MT, 4, 128], BF16, name="xt", tag="xt")
            wv_sb = [pw.tile([128, DIM], BF16, name=f"wv{i}", tag=f"wv{i}") for i in range(4)]
            for i in range(4):
                nc.gpsimd.dma_start(wv_sb[i][:], wv[128 * i:128 * (i + 1), :])

            ei = 0

            def evict_bias(dst, src, bcol):
                nonlocal ei
                if ei % 2 == 0:
                    nc.scalar.activation(dst, src, AF.Identity,
                                         bias=biaqk_sb[:, bcol:bcol + 1])
                else:
                    nc.vector.tensor_scalar_add(dst, src,
                                                biaqk_sb[:, bcol:bcol + 1])
                ei += 1

            def evict(dst, src):
                nonlocal ei
                if ei % 2 == 0:
                    nc.scalar.copy(dst, src)
                else:
                    nc.vector.tensor_copy(dst, src)
                ei += 1

            # ================= phase A: x^T + QKV projection =================
            with tc.tile_pool(name="pxt", bufs=1) as pxt:
                # qt stays f32 so qhat is a single rounding f32 -> fp8
                qt_sb = [pxt.tile([128, NLOC], F32, name=f"qt{i}", tag=f"qt{i}")
                         for i in range(4)]
                wq_sb = [pxt.tile([128, DIM], BF16, name=f"wq{i}", tag=f"wq{i}") for i in range(4)]
                wk_sb = [pxt.tile([128, DIM], BF16, name=f"wk{i}", tag=f"wk{i}") for i in range(4)]
                for i in range(4):
                    nc.gpsimd.dma_start(wq_sb[i][:], wq[128 * i:128 * (i + 1), :])
                    nc.gpsimd.dma_start(wk_sb[i][:], wk[128 * i:128 * (i + 1), :])

                # HAM warm-up: full-depth dummy matmuls while the x DMA runs
                # (rank-1 matmuls don't register as PE activity for the HAM).
                junk = pxt.tile([128, 512], BF16, name="junk", tag="junk")
                nc.any.memset(junk[:], 0.5)
                wps = psB.tile([128, 512], F32, name="warm", tag="pss")
                for _ in range(24):
                    nc.tensor.matmul(wps[:], junk[:, 0:128], junk[:],
                                     start=True, stop=True)

                def load_pair(pr):
                    # x arrives bf16 from the host: plain hardware-DGE load
                    xb2 = pxt.tile([128, 2, DIM], BF16, name="xb", tag="xb", bufs=2)
                    nc.sync.dma_start(xb2[:, 0, :], x[256 * pr:256 * pr + 128, :])
                    nc.sync.dma_start(xb2[:, 1, :], x[256 * pr + 128:256 * (pr + 1), :])
                    # junk matmul reading the freshly loaded pair: keeps the
                    # PE HAM-warm through the input-staging window
                    nc.tensor.matmul(wps[:], junk[:, 0:128], xb2[:, 0, :],
                                     start=True, stop=True)
                    nc.sync.dma_start_transpose(xt_sb[:, 2 * pr:2 * (pr + 1), :, :], xb2[:])

                def kproj(mch):
                    for rc in range(4):
                        ps = psB.tile([128, 512], F32, name="psk", tag="pss")
                        for j in range(4):
                            rhs = xt_sb[:, 4 * mch:4 * (mch + 1), j, :]
                            nc.tensor.matmul(ps[:], wk_sb[j][:, 128 * rc:128 * (rc + 1)],
                                             rhs, start=(j == 0), stop=(j == 3))
                        evict_bias(kt_dst(rc, 512 * mch, 512 * (mch + 1)), ps[:], 4 + rc)

                def qproj(nch):
                    for rc in range(4):
                        ps = psB.tile([128, 512], F32, name="psq", tag="pss")
                        for j in range(4):
                            rhs = xt_sb[:, 4 * nch:4 * (nch + 1), j, :]
                            nc.tensor.matmul(ps[:], wq_sb[j][:, 128 * rc:128 * (rc + 1)],
                                             rhs, start=(j == 0), stop=(j == 3))
                        evict_bias(qt_sb[rc][:, 512 * nch:512 * (nch + 1)], ps[:], rc)
                        # th1-fused interleaved Qhat build for this chunk
                        qhr = qhat_dst(rc).rearrange("p (j g n) -> p j g n",
                                                     g=8, n=NG)
                        qtr = qt_sb[rc][:, 512 * nch:512 * (nch + 1)].rearrange(
                            "p (j n) -> p j n", n=NG)
                        for g in range(8):
                            nc.vector.tensor_scalar_mul(
                                qhr[:, 32 * nch:32 * (nch + 1), g, :], qtr,
                                th1s_sb[:, 8 * rc + g:8 * rc + g + 1])

                # pipeline: transposes flow while projections consume them
                load_pair(0); load_pair(1)
                kproj(0)
                load_pair(2); load_pair(3)
                kproj(1)
                qproj(0)
                load_pair(4); load_pair(5)
                qproj(1)
                kproj(2)
                load_pair(6); load_pair(7)
                kproj(3)

                def vproj(mt):
                    # V bias is folded into the output-projection bias on host
                    ps = psB.tile([128, 512], F32, name="psv", tag="pss")
                    for j in range(4):
                        nc.tensor.matmul(ps[:], xt_sb[:, mt, j, :], wv_sb[j][:],
                                         start=(j == 0), stop=(j == 3))
                    evict(v_sb[:, mt, :], ps[:])
                vfill = [lambda mt=mt: vproj(mt) for mt in range(MT)]

            # ================= phase B: attention main loop =================
            # th1 mix folded into the score matmuls (Qhat columns th1-scaled,
            # head-interleaved): scores land directly in interleaved PSUM.
            # Emission order per k: fscore(k+2) [PE runway], m2(k), l2prep(k+2)
            # [vector, behind m2(k)'s casts], av/vfill fillers.
            with tc.tile_pool(name="pk", bufs=1) as pk:
                st = {}

                def mk_fscore(t):
                    st[t] = {"u": {}, "zz": {},
                             "at": pk.tile([128, 8, 16, 128], BF16, name="at2",
                                           tag="at", bufs=2)}

                    def fscore(j):
                        u = pk.tile([128, N], BF16, name="u", tag="u", bufs=3)
                        zz = pk.tile([128, 68], F32, name="zz", tag="zz", bufs=3)
                        st[t]["u"][j] = u
                        st[t]["zz"][j] = zz
                        for half in range(2):
                            ps = psA.tile([128, 1024], F32, name="psm1", tag="psm")
                            for mc in range(2):
                                m0 = 1024 * half + 512 * mc
                                if FP8:
                                    for pr in range(2):
                                        nc.tensor.matmul(
                                            ps[:, 512 * mc:512 * (mc + 1)],
                                            qhat_sb[pr][:, :, 128 * (8 * t + j):
                                                        128 * (8 * t + j) + 128],
                                            kt_sb[pr][:, :, m0:m0 + 512],
                                            start=(pr == 0), stop=(pr == 1),
                                            perf_mode=DR)
                                else:
                                    for rc in range(4):
                                        nc.tensor.matmul(
                                            ps[:, 512 * mc:512 * (mc + 1)],
                                            qhat_sb[rc][:, 128 * (8 * t + j):
                                                        128 * (8 * t + j) + 128],
                                            kt_sb[rc][:, m0:m0 + 512],
                                            start=(rc == 0), stop=(rc == 3))
                            nc.scalar.activation(u[:, 1024 * half:1024 * (half + 1)],
                                                 ps[:], AF.Exp, scale=SCALE,
                                                 accum_out=zz[:, half:half + 1])
                    return fscore

                def mk_l2prep(t):
                    def l2prep(j):
                        zz = st[t]["zz"][j]
                        nc.vector.tensor_add(zz[:, 2:3], zz[:, 0:1], zz[:, 1:2])
                        nc.vector.reciprocal(zz[:, 3:4], zz[:, 2:3])
                        l2 = zz[:, 4:68].bitcast(BF16)
                        nc.vector.tensor_scalar_mul(l2, t2_sb[:], zz[:, 3:4])
                    return l2prep

                def mk_m2(t):
                    at_h = st[t]["at"]

                    def m2(j):
                        u = st[t]["u"][j]
                        l2 = st[t]["zz"][j][:, 4:68].bitcast(BF16)
                        a = pk.tile([128, N], BF16, name="a", tag="a", bufs=4)
                        for half in range(2):
                            ps = psA.tile([128, 1024], F32, name="psm2", tag="psm")
                            for mc in range(2):
                                m0 = 1024 * half + 512 * mc
                                nc.tensor.matmul(ps[:, 512 * mc:512 * (mc + 1)],
                                                 l2, u[:, m0:m0 + 512],
                                                 start=True, stop=True)
                            # balance the big evicts across DVE and ACT
                            # (ACT also carries the exps: give it 1 in 4)
                            if half == 1 and j % 2 == 0:
                                nc.scalar.copy(
                                    a[:, 1024 * half:1024 * (half + 1)], ps[:])
                            else:
                                nc.vector.tensor_copy(
                                    a[:, 1024 * half:1024 * (half + 1)], ps[:])
                        nc.sync.dma_start_transpose(at_h[:, j, :, :], a[:])
                    return m2

                def mk_av(tv):
                    at_h = st[tv]["at"]
                    otb = pk.tile([128, 4, 128], BF16, name="otb", tag="otb", bufs=2)
                    st[tv]["otb"] = otb

                    def av(rc):
                        # the two heads of the pair run in the two 64-col
                        # halves of the PE array concurrently (col tiling)
                        ps = psB.tile([128, 128], F32, name="psav", tag="pss")
                        for mchunk in range(16):
                            for gi in range(2):
                                g = 2 * rc + gi
                                rhs = at_h[:, :, mchunk, NG * g:NG * (g + 1)]
                                nc.tensor.matmul(
                                    ps[64 * gi:64 * (gi + 1), :],
                                    v_sb[:, mchunk, 64 * g:64 * (g + 1)],
                                    rhs, start=(mchunk == 0),
                                    stop=(mchunk == 15),
                                    tile_position=(0, 64 * gi))
                        nc.vector.tensor_copy(otb[:, rc, :], ps[:])
                    return av

                def emit_outproj(tv):
                    otb = st[tv]["otb"]
                    ps = psB.tile([128, DIM], F32, name="pso", tag="pss")
                    for rc in range(4):
                        nc.tensor.matmul(ps[:], otb[:, rc, :], wo_sb[rc][:],
                                         start=(rc == 0), stop=False)
                    nc.tensor.matmul(ps[:], ones_sb[:, 0:128],
                                     bia_sb[0:1, 3 * DIM:4 * DIM],
                                     start=False, stop=True)
                    yt = pk.tile([128, DIM], F32, name="yt", tag="yt", bufs=2)
                    nc.vector.tensor_copy(yt[:], ps[:])
                    nc.sync.dma_start(y[128 * tv:128 * (tv + 1), :], yt[:])
                    del st[tv]

                for i in range(NT + 1):
                    tm = i if i < NT else None
                    tv = i - 1 if i >= 1 else None
                    fscore = mk_fscore(tm) if tm is not None else None
                    l2prep = mk_l2prep(tm) if tm is not None else None
                    m2 = mk_m2(tm) if tm is not None else None
                    av = mk_av(tv) if tv is not None else None
                    if tm is not None:
                        fscore(0)
                        fscore(1)
                        l2prep(0)
                        l2prep(1)
                    for k in range(8):
                        if tm is not None:
                            if k + 2 < 8:
                                fscore(k + 2)
                            m2(k)
                            if k + 2 < 8:
                                l2prep(k + 2)
                        if vfill and tv is None:
                            vfill.pop(0)()
                            if vfill:
                                vfill.pop(0)()
                            if vfill and k % 2 == 1:
                                vfill.pop(0)()
                    while vfill and tv is None:
                        vfill.pop(0)()
                    # @V for the previous tile at iteration end: its at
                    # transposes had a full iteration to drain, so the PE
                    # never head-of-line blocks on the DMA fabric here
                    if tv is not None:
                        for rc in range(4):
                            av(rc)
                        emit_outproj(tv)

    nc.compile()
    return nc


_NC_CACHE = None


def _get_nc():
    global _NC_CACHE
    if _NC_CACHE is None:
        _NC_CACHE = build_nc()
    return _NC_CACHE


def _host_prep(w_qkv, b_qkv, th1, th2, w_out, b_out):
    bf = ml_dtypes.bfloat16
    w_qkv = np.asarray(w_qkv, dtype=np.float32)
    wq = w_qkv[:, 0:DIM].astype(bf)
    wk = w_qkv[:, DIM:2 * DIM].astype(bf)
    wv = w_qkv[:, 2 * DIM:3 * DIM].astype(bf)
    wo = np.asarray(w_out, dtype=np.float32).astype(bf)
    th1 = np.asarray(th1, dtype=np.float32)
    th2 = np.asarray(th2, dtype=np.float32)
    # th1 spread for fused scores: th1s[p, rc*8+g] = th1[g, rc*2 + p//64]
    th1s = np.zeros((128, 32), dtype=np.float32)
    for rc in range(4):
        for g in range(8):
            for p in range(128):
                th1s[p, 8 * rc + g] = th1[g, rc * 2 + p // 64]
    # mix2 template, (g, n16) partition order:
    # t2t[g*16+n16, g2*16+n16] = th2[g2, g]
    t2t = np.zeros((128, 128), dtype=np.float32)
    for n16 in range(NG):
        t2t[n16::NG, n16::NG] = th2.T
    bqkv = np.asarray(b_qkv, dtype=np.float32)
    bia = np.zeros((1, 4 * DIM), dtype=np.float32)
    bia[0, 0:3 * DIM] = bqkv
    # V bias folds into the output bias: row-sums of A' are th2 row-sums,
    # so attn' @ (1 b_v^T) contributes (sum_h th2[g,h]) * b_v[g-block].
    bv = bqkv[2 * DIM:3 * DIM].reshape(H, DH)
    cg = th2.sum(axis=1)
    bv_fold = (cg[:, None] * bv).reshape(H * DH) @ np.asarray(w_out, np.float32)
    bia[0, 3 * DIM:] = np.asarray(b_out, dtype=np.float32) + bv_fold
    biaqk = np.zeros((128, 8), dtype=np.float32)
    for rc in range(4):
        biaqk[:, rc] = bqkv[128 * rc:128 * (rc + 1)]
        biaqk[:, 4 + rc] = bqkv[DIM + 128 * rc:DIM + 128 * (rc + 1)]
    return (wq, wk, wv, wo, th1s, t2t.astype(bf), bia.astype(bf), biaqk)


def _in_maps(x, w_qkv, b_qkv, th1, th2, w_out, b_out):
    x = np.asarray(x, dtype=np.float32)
    wq, wk, wv, wo, th1s, t2t, bia, biaqk = _host_prep(
        w_qkv, b_qkv, th1, th2, w_out, b_out)
    in_maps = []
    for c in range(NCORES):
        b, half = c // 2, c % 2
        # query half first; key order is attention-invariant
        xp = np.concatenate(
            [x[b, NLOC * half:NLOC * (half + 1), :],
             x[b, NLOC * (1 - half):NLOC * (2 - half), :]], axis=0)
        in_maps.append({
            "x": np.ascontiguousarray(xp.astype(ml_dtypes.bfloat16)),
            "wq": wq, "wk": wk, "wv": wv, "wo": wo,
            "th1s": th1s, "t2t": t2t, "bia": bia, "biaqk": biaqk,
        })
    return in_maps


def kernel(x, w_qkv, b_qkv, th1, th2, w_out, b_out):
    nc = _get_nc()
    in_maps = _in_maps(x, w_qkv, b_qkv, th1, th2, w_out, b_out)
    res = run_bass_kernel_spmd(nc, in_maps, core_ids=list(range(NCORES)))
    out = np.empty((B, N, DIM), dtype=np.float32)
    for c in range(NCORES):
        b, half = c // 2, c % 2
        out[b, NLOC * half:NLOC * (half + 1), :] = res.results[c]["y"]
    return out


# revision 17
# speedup vs baseline: 1.1750x; 1.1750x over previous
"""Talking-heads attention kernel for Trainium2, 8 NeuronCores.

Problem: B=4, N=2048, DIM=512, H=8, DH=64 talking-heads attention
(qkv proj -> per-head scores -> th1 head-mix -> softmax -> th2 head-mix
 -> attn @ v -> out proj).

Sharding: data-parallel over (batch, query-half): core c handles batch c//2,
query rows [1024*(c%2), 1024*(c%2)+1024).  Communication-free.  The host
permutes each core's x so its query half comes first; key order is
attention-invariant, so K/V use the permuted order untouched.

Device pipeline per core:
  1. x -> bf16 -> xbar-transpose (pairs of tiles per transpose) -> x^T;
     QKV projection on TensorE.  K^T and the th1-fused interleaved Qhat are
     evicted straight to fp8e4 (scores run fp8 DoubleRow, 256-deep chains);
     the softmax scale 1/sqrt(dh) is applied inside the exp activation.
  2. Dummy warm-up matmuls during the input DMA keep the PE HAM clock at
     full rate; V-projection matmuls are deferred into the idle @V slots of
     main-loop iteration 0 (vfill).
  3. Per 128-query tile: fused scores land in interleaved PSUM; exp on
     ScalarE with fused per-row accumulation; softmax division folded into
     the second mix's stationary weights (th2[g,h]/Z row scaling).
  4. The Z->reciprocal->l2 chain is emitted right after each m2 (two
     iterations ahead of use) so it never head-of-line blocks the Vector
     queue; fscore(k+2) is emitted before m2(k) so the PE always has
     runway while that chain completes.
  5. mix2 output is xbar-DMA-transposed to key-major A^T, attn@V runs with
     V as the stationary operand, followed by the output projection.
"""

import sys

sys.path.insert(0, "/opt/trn_rl_repo")

import numpy as np
import ml_dtypes

import concourse.bass as bass
from concourse import bacc
import concourse.mybir as mybir
import concourse.tile as tile
from concourse.bass_utils import run_bass_kernel_spmd

BF16 = mybir.dt.bfloat16
F32 = mybir.dt.float32
F8 = mybir.dt.float8e4
AF = mybir.ActivationFunctionType
DR = mybir.MatmulPerfMode.DoubleRow

FP8 = True            # fp8e4 + DoubleRow score matmuls

B, N, DIM = 4, 2048, 512
H, DH = 8, 64
NCORES = 8
NLOC = N // 2          # query rows per core
NT = NLOC // 128       # 8 query tiles per core
MT = N // 128          # 16 key chunks
NG = 16                # n16 group size in interleaved tiles
SCALE = DH ** -0.5


def build_nc():
    nc = bacc.Bacc()

    x = nc.declare_dram_parameter("x", [N, DIM], BF16, isOutput=False)
    wq = nc.declare_dram_parameter("wq", [DIM, DIM], BF16, isOutput=False)
    wk = nc.declare_dram_parameter("wk", [DIM, DIM], BF16, isOutput=False)
    wv = nc.declare_dram_parameter("wv", [DIM, DIM], BF16, isOutput=False)
    wo = nc.declare_dram_parameter("wo", [DIM, DIM], BF16, isOutput=False)
    th1s = nc.declare_dram_parameter("th1s", [128, 32], F32, isOutput=False)
    t2t = nc.declare_dram_parameter("t2t", [128, 128], BF16, isOutput=False)
    # rows: bq, bk, bv, bo (bq unscaled; scale applied in exp)
    bia = nc.declare_dram_parameter("bia", [1, 4 * DIM], BF16, isOutput=False)
    # per-partition q/k bias: col rc -> bq[128rc+p], col 4+rc -> bk[128rc+p]
    biaqk = nc.declare_dram_parameter("biaqk", [128, 8], F32, isOutput=False)
    y = nc.declare_dram_parameter("y", [NLOC, DIM], F32, isOutput=True)

    SD = F8 if FP8 else BF16

    with tile.TileContext(nc) as tc:
        with (
            tc.tile_pool(name="pw", bufs=1) as pw,      # persistent across phases
            tc.tile_pool(name="psA", bufs=3, space="PSUM") as psA,  # [128,1024] mixes
            tc.tile_pool(name="psB", bufs=2, space="PSUM") as psB,  # [128,512] scores/proj/@V
        ):
            # persistent small tensors
            wo_sb = [pw.tile([128, DIM], BF16, name=f"wo{i}", tag=f"wo{i}") for i in range(4)]
            for i in range(4):
                nc.scalar.dma_start(wo_sb[i][:], wo[128 * i:128 * (i + 1), :])
            th1s_sb = pw.tile([128, 32], F32, name="th1s", tag="th1s")
            t2_sb = pw.tile([128, 128], BF16, name="t2", tag="t2")
            nc.scalar.dma_start(th1s_sb[:], th1s[:])
            nc.scalar.dma_start(t2_sb[:], t2t[:])
            bia_sb = pw.tile([1, 4 * DIM], BF16, name="bia", tag="bia")
            nc.scalar.dma_start(bia_sb[:], bia[:])
            biaqk_sb = pw.tile([128, 8], F32, name="biaqk", tag="biaqk")
            nc.scalar.dma_start(biaqk_sb[:], biaqk[:])
            ones_sb = pw.tile([1, DIM], BF16, name="ones", tag="ones")
            nc.any.memset(ones_sb[:], 1.0)

            # persistent activations: th1-fused interleaved Qhat, K^T, V.
            # qhat[p=(h,d) in chunk rc, col (j, g, n16)] = th1[g,h] * Q^T[(h,d), n]
            # with n = 16*j + n16 (local query index).  For fp8 the four rc
            # chunks live as two DoubleRow pair tiles [128, ko=2, cols].
            if FP8:
                qhat_sb = [pw.tile([128, 2, NLOC * 8], SD, name=f"qh{i}", tag=f"qh{i}")
                           for i in range(2)]
                kt_sb = [pw.tile([128, 2, N], SD, name=f"kt{i}", tag=f"kt{i}")
                         for i in range(2)]

                def qhat_dst(rc):
                    return qhat_sb[rc // 2][:, rc % 2, :]

                def kt_dst(rc, c0, c1):
                    return kt_sb[rc // 2][:, rc % 2, c0:c1]
            else:
                qhat_sb = [pw.tile([128, NLOC * 8], SD, name=f"qh{i}", tag=f"qh{i}")
                           for i in range(4)]
                kt_sb = [pw.tile([128, N], SD, name=f"kt{i}", tag=f"kt{i}")
                         for i in range(4)]

                def qhat_dst(rc):
                    return qhat_sb[rc][:]

                def kt_dst(rc, c0, c1):
                    return kt_sb[rc][:, c0:c1]

            v_sb = pw.tile([128, MT, DIM], BF16, name="v", tag="v")
            # xt/wv persist into iteration 0 (vfill) so they live in pw
            # xt_sb[p, t, j, f]: dim = j*128+p, key row m = t*128+f
            xt_sb = pw.tile([128, MT, 4, 128], BF16, name="xt", tag="xt")
            wv_sb = [pw.tile([128, DIM], BF16, name=f"wv{i}", tag=f"wv{i}") for i in range(4)]
            for i in range(4):
                nc.scalar.dma_start(wv_sb[i][:], wv[128 * i:128 * (i + 1), :])

            ei = 0

            def evict_bias(dst, src, bcol):
                nonlocal ei
                if ei % 2 == 0:
                    nc.scalar.activation(dst, src, AF.Identity,
                                         bias=biaqk_sb[:, bcol:bcol + 1])
                else:
                    nc.vector.tensor_scalar_add(dst, src,
                                                biaqk_sb[:, bcol:bcol + 1])
                ei += 1

            def evict(dst, src):
                nonlocal ei
                if ei % 2 == 0:
                    nc.scalar.copy(dst, src)
                else:
                    nc.vector.tensor_copy(dst, src)
                ei += 1

            # ================= phase A: x^T + QKV projection =================
            with tc.tile_pool(name="pxt", bufs=1) as pxt:
                # qt stays f32 so qhat is a single rounding f32 -> fp8
                qt_sb = [pxt.tile([128, NLOC], F32, name=f"qt{i}", tag=f"qt{i}")
                         for i in range(4)]
                wq_sb = [pxt.tile([128, DIM], BF16, name=f"wq{i}", tag=f"wq{i}") for i in range(4)]
                wk_sb = [pxt.tile([128, DIM], BF16, name=f"wk{i}", tag=f"wk{i}") for i in range(4)]
                for i in range(4):
                    nc.sync.dma_start(wk_sb[i][:], wk[128 * i:128 * (i + 1), :])
                for i in range(4):
                    nc.scalar.dma_start(wq_sb[i][:], wq[128 * i:128 * (i + 1), :])

                # HAM warm-up: full-depth dummy matmuls while the x DMA runs
                # (rank-1 matmuls don't register as PE activity for the HAM).
                junk = pxt.tile([128, 512], BF16, name="junk", tag="junk")
                nc.any.memset(junk[:], 0.5)
                wps = psB.tile([128, 512], F32, name="warm", tag="pss")
                for _ in range(12):
                    nc.tensor.matmul(wps[:], junk[:, 0:128], junk[:],
                                     start=True, stop=True)

                def load_pair(pr):
                    # x arrives bf16 from the host: plain hardware-DGE load
                    xb2 = pxt.tile([128, 2, DIM], BF16, name="xb", tag="xb", bufs=2)
                    nc.sync.dma_start(xb2[:, 0, :], x[256 * pr:256 * pr + 128, :])
                    nc.sync.dma_start(xb2[:, 1, :], x[256 * pr + 128:256 * (pr + 1), :])
                    # junk matmul reading the freshly loaded pair: keeps the
                    # PE HAM-warm through the input-staging window
                    nc.tensor.matmul(wps[:], junk[:, 0:128], xb2[:, 0, :],
                                     start=True, stop=True)
                    nc.sync.dma_start_transpose(xt_sb[:, 2 * pr:2 * (pr + 1), :, :], xb2[:])

                def kproj(mch):
                    for rc in range(4):
                        ps = psB.tile([128, 512], F32, name="psk", tag="pss")
                        for j in range(4):
                            rhs = xt_sb[:, 4 * mch:4 * (mch + 1), j, :]
                            nc.tensor.matmul(ps[:], wk_sb[j][:, 128 * rc:128 * (rc + 1)],
                                             rhs, start=(j == 0), stop=(j == 3))
                        evict_bias(kt_dst(rc, 512 * mch, 512 * (mch + 1)), ps[:], 4 + rc)

                def qproj(nch):
                    for rc in range(4):
                        ps = psB.tile([128, 512], F32, name="psq", tag="pss")
                        for j in range(4):
                            rhs = xt_sb[:, 4 * nch:4 * (nch + 1), j, :]
                            nc.tensor.matmul(ps[:], wq_sb[j][:, 128 * rc:128 * (rc + 1)],
                                             rhs, start=(j == 0), stop=(j == 3))
                        evict_bias(qt_sb[rc][:, 512 * nch:512 * (nch + 1)], ps[:], rc)
                        # th1-fused interleaved Qhat build for this chunk
                        qhr = qhat_dst(rc).rearrange("p (j g n) -> p j g n",
                                                     g=8, n=NG)
                        qtr = qt_sb[rc][:, 512 * nch:512 * (nch + 1)].rearrange(
                            "p (j n) -> p j n", n=NG)
                        for g in range(8):
                            nc.vector.tensor_scalar_mul(
                                qhr[:, 32 * nch:32 * (nch + 1), g, :], qtr,
                                th1s_sb[:, 8 * rc + g:8 * rc + g + 1])

                # pipeline: transposes flow while projections consume them
                load_pair(0); load_pair(1)
                kproj(0)
                load_pair(2); load_pair(3)
                kproj(1)
                qproj(0)
                load_pair(4); load_pair(5)
                qproj(1)
                kproj(2)
                load_pair(6); load_pair(7)
                kproj(3)

                def vproj(mt):
                    # V bias is folded into the output-projection bias on host
                    ps = psB.tile([128, 512], F32, name="psv", tag="pss")
                    for j in range(4):
                        nc.tensor.matmul(ps[:], xt_sb[:, mt, j, :], wv_sb[j][:],
                                         start=(j == 0), stop=(j == 3))
                    evict(v_sb[:, mt, :], ps[:])
                vfill = [lambda mt=mt: vproj(mt) for mt in range(MT)]

            # ================= phase B: attention main loop =================
            # th1 mix folded into the score matmuls (Qhat columns th1-scaled,
            # head-interleaved): scores land directly in interleaved PSUM.
            # Emission order per k: fscore(k+2) [PE runway], m2(k), l2prep(k+2)
            # [vector, behind m2(k)'s casts], av/vfill fillers.
            with tc.tile_pool(name="pk", bufs=1) as pk:
                st = {}

                def mk_fscore(t):
                    st[t] = {"u": {}, "zz": {},
                             "at": pk.tile([128, 8, 16, 128], BF16, name="at2",
                                           tag="at", bufs=2)}

                    def fscore(j):
                        u = pk.tile([128, N], BF16, name="u", tag="u", bufs=3)
                        zz = pk.tile([128, 68], F32, name="zz", tag="zz", bufs=3)
                        st[t]["u"][j] = u
                        st[t]["zz"][j] = zz
                        for half in range(2):
                            ps = psA.tile([128, 1024], F32, name="psm1", tag="psm")
                            for mc in range(2):
                                m0 = 1024 * half + 512 * mc
                                if FP8:
                                    for pr in range(2):
                                        nc.tensor.matmul(
                                            ps[:, 512 * mc:512 * (mc + 1)],
                                            qhat_sb[pr][:, :, 128 * (8 * t + j):
                                                        128 * (8 * t + j) + 128],
                                            kt_sb[pr][:, :, m0:m0 + 512],
                                            start=(pr == 0), stop=(pr == 1),
                                            perf_mode=DR)
                                else:
                                    for rc in range(4):
                                        nc.tensor.matmul(
                                            ps[:, 512 * mc:512 * (mc + 1)],
                                            qhat_sb[rc][:, 128 * (8 * t + j):
                                                        128 * (8 * t + j) + 128],
                                            kt_sb[rc][:, m0:m0 + 512],
                                            start=(rc == 0), stop=(rc == 3))
                            nc.scalar.activation(u[:, 1024 * half:1024 * (half + 1)],
                                                 ps[:], AF.Exp, scale=SCALE,
                                                 accum_out=zz[:, half:half + 1])
                    return fscore

                def mk_l2prep(t):
                    def l2prep(j):
                        zz = st[t]["zz"][j]
                        nc.vector.tensor_add(zz[:, 2:3], zz[:, 0:1], zz[:, 1:2])
                        nc.vector.reciprocal(zz[:, 3:4], zz[:, 2:3])
                        l2 = zz[:, 4:68].bitcast(BF16)
                        nc.vector.tensor_scalar_mul(l2, t2_sb[:], zz[:, 3:4])
                    return l2prep

                def mk_m2(t):
                    at_h = st[t]["at"]

                    def m2(j):
                        u = st[t]["u"][j]
                        l2 = st[t]["zz"][j][:, 4:68].bitcast(BF16)
                        a = pk.tile([128, N], BF16, name="a", tag="a", bufs=4)
                        for half in range(2):
                            ps = psA.tile([128, 1024], F32, name="psm2", tag="psm")
                            for mc in range(2):
                                m0 = 1024 * half + 512 * mc
                                nc.tensor.matmul(ps[:, 512 * mc:512 * (mc + 1)],
                                                 l2, u[:, m0:m0 + 512],
                                                 start=True, stop=True)
                            # balance the big evicts across DVE and ACT
                            # (ACT also carries the exps: give it 1 in 4)
                            if half == 1 and j % 2 == 0:
                                nc.scalar.copy(
                                    a[:, 1024 * half:1024 * (half + 1)], ps[:])
                            else:
                                nc.vector.tensor_copy(
                                    a[:, 1024 * half:1024 * (half + 1)], ps[:])
                        nc.sync.dma_start_transpose(at_h[:, j, :, :], a[:])
                    return m2

                def mk_av(tv):
                    at_h = st[tv]["at"]
                    otb = pk.tile([128, 4, 128], BF16, name="otb", tag="otb", bufs=2)
                    st[tv]["otb"] = otb

                    def av(rc):
                        # the two heads of the pair run in the two 64-col
                        # halves of the PE array concurrently (col tiling)
                        ps = psB.tile([128, 128], F32, name="psav", tag="pss")
                        for mchunk in range(16):
                            for gi in range(2):
                                g = 2 * rc + gi
                                rhs = at_h[:, :, mchunk, NG * g:NG * (g + 1)]
                                nc.tensor.matmul(
                                    ps[64 * gi:64 * (gi + 1), :],
                                    v_sb[:, mchunk, 64 * g:64 * (g + 1)],
                                    rhs, start=(mchunk == 0),
                                    stop=(mchunk == 15),
                                    tile_position=(0, 64 * gi))
                        nc.vector.tensor_copy(otb[:, rc, :], ps[:])
                    return av

                def emit_outproj(tv):
                    otb = st[tv]["otb"]
                    ps = psB.tile([128, DIM], F32, name="pso", tag="pss")
                    for rc in range(4):
                        nc.tensor.matmul(ps[:], otb[:, rc, :], wo_sb[rc][:],
                                         start=(rc == 0), stop=False)
                    nc.tensor.matmul(ps[:], ones_sb[:, 0:128],
                                     bia_sb[0:1, 3 * DIM:4 * DIM],
                                     start=False, stop=True)
                    yt = pk.tile([128, DIM], F32, name="yt", tag="yt", bufs=2)
                    nc.vector.tensor_copy(yt[:], ps[:])
                    nc.sync.dma_start(y[128 * tv:128 * (tv + 1), :], yt[:])
                    del st[tv]

                for i in range(NT + 1):
                    tm = i if i < NT else None
                    tv = i - 1 if i >= 1 else None
                    fscore = mk_fscore(tm) if tm is not None else None
                    l2prep = mk_l2prep(tm) if tm is not None else None
                    m2 = mk_m2(tm) if tm is not None else None
                    av = mk_av(tv) if tv is not None else None
                    if tm is not None:
                        fscore(0)
                        fscore(1)
                        l2prep(0)
                        l2prep(1)
                    for k in range(8):
                        if tm is not None:
                            if k + 2 < 8:
                                fscore(k + 2)
                            m2(k)
                            if k + 2 < 8:
                                l2prep(k + 2)
                        if vfill and tv is None:
                            vfill.pop(0)()
                            if vfill:
                                vfill.pop(0)()
                            if vfill and k % 2 == 1:
                                vfill.pop(0)()
                    while vfill and tv is None:
                        vfill.pop(0)()
                    # @V for the previous tile at iteration end: its at
                    # transposes had a full iteration to drain, so the PE
                    # never head-of-line blocks on the DMA fabric here
                    if tv is not None:
                        for rc in range(4):
                            av(rc)
                        emit_outproj(tv)

    nc.compile()
    return nc


_NC_CACHE = None


def _get_nc():
    global _NC_CACHE
    if _NC_CACHE is None:
        _NC_CACHE = build_nc()
    return _NC_CACHE


def _host_prep(w_qkv, b_qkv, th1, th2, w_out, b_out):
    bf = ml_dtypes.bfloat16
    w_qkv = np.asarray(w_qkv, dtype=np.float32)
    wq = w_qkv[:, 0:DIM].astype(bf)
    wk = w_qkv[:, DIM:2 * DIM].astype(bf)
    wv = w_qkv[:, 2 * DIM:3 * DIM].astype(bf)
    wo = np.asarray(w_out, dtype=np.float32).astype(bf)
    th1 = np.asarray(th1, dtype=np.float32)
    th2 = np.asarray(th2, dtype=np.float32)
    # th1 spread for fused scores: th1s[p, rc*8+g] = th1[g, rc*2 + p//64]
    th1s = np.zeros((128, 32), dtype=np.float32)
    for rc in range(4):
        for g in range(8):
            for p in range(128):
                th1s[p, 8 * rc + g] = th1[g, rc * 2 + p // 64]
    # mix2 template, (g, n16) partition order:
    # t2t[g*16+n16, g2*16+n16] = th2[g2, g]
    t2t = np.zeros((128, 128), dtype=np.float32)
    for n16 in range(NG):
        t2t[n16::NG, n16::NG] = th2.T
    bqkv = np.asarray(b_qkv, dtype=np.float32)
    bia = np.zeros((1, 4 * DIM), dtype=np.float32)
    bia[0, 0:3 * DIM] = bqkv
    # V bias folds into the output bias: row-sums of A' are th2 row-sums,
    # so attn' @ (1 b_v^T) contributes (sum_h th2[g,h]) * b_v[g-block].
    bv = bqkv[2 * DIM:3 * DIM].reshape(H, DH)
    cg = th2.sum(axis=1)
    bv_fold = (cg[:, None] * bv).reshape(H * DH) @ np.asarray(w_out, np.float32)
    bia[0, 3 * DIM:] = np.asarray(b_out, dtype=np.float32) + bv_fold
    biaqk = np.zeros((128, 8), dtype=np.float32)
    for rc in range(4):
        biaqk[:, rc] = bqkv[128 * rc:128 * (rc + 1)]
        biaqk[:, 4 + rc] = bqkv[DIM + 128 * rc:DIM + 128 * (rc + 1)]
    return (wq, wk, wv, wo, th1s, t2t.astype(bf), bia.astype(bf), biaqk)


def _in_maps(x, w_qkv, b_qkv, th1, th2, w_out, b_out):
    x = np.asarray(x, dtype=np.float32)
    wq, wk, wv, wo, th1s, t2t, bia, biaqk = _host_prep(
        w_qkv, b_qkv, th1, th2, w_out, b_out)
    in_maps = []
    for c in range(NCORES):
        b, half = c // 2, c % 2
        # query half first; key order is attention-invariant
        xp = np.concatenate(
            [x[b, NLOC * half:NLOC * (half + 1), :],
             x[b, NLOC * (1 - half):NLOC * (2 - half), :]], axis=0)
        in_maps.append({
            "x": np.ascontiguousarray(xp.astype(ml_dtypes.bfloat16)),
            "wq": wq, "wk": wk, "wv": wv, "wo": wo,
            "th1s": th1s, "t2t": t2t, "bia": bia, "biaqk": biaqk,
        })
    return in_maps


def kernel(x, w_qkv, b_qkv, th1, th2, w_out, b_out):
    nc = _get_nc()
    in_maps = _in_maps(x, w_qkv, b_qkv, th1, th2, w_out, b_out)
    res = run_bass_kernel_spmd(nc, in_maps, core_ids=list(range(NCORES)))
    out = np.empty((B, N, DIM), dtype=np.float32)
    for c in range(NCORES):
        b, half = c // 2, c % 2
        out[b, NLOC * half:NLOC * (half + 1), :] = res.results[c]["y"]
    return out
